# revision 46
# baseline (speedup 1.0000x reference)
"""Multi-head attention forward on 8 Trainium2 NeuronCores.

Problem: x [2,2048,1024], weights wq/wk/wv/wo [1024,1024] (torch Linear
layout, y = x @ W.T), 16 heads, head_dim 64, fp32.

Sharding: core c handles batch b = c//4 and head group g = c%4 (heads
4g..4g+3, i.e. 256 output dims of wq/wk/wv and 256 input dims of wo).
Each core computes a partial output [2048, 1024]; the host sums the 4
partials per batch (the reduce is host-side, no collectives).

On-core plan (v5, 239us vs the 335us v2 baseline):
  - All inputs are host-cast to bf16; DMA traffic halves vs fp32 and
    every matmul runs bf16 (full PE rate + fast weight load).  Few big
    DMA triggers (~600ns each); the first xT chunk is split across the
    sync+scalar queues so the projection phase starts earliest.
  - EVERY matmul runs in the full (128,128) array mode.  Mixing array
    tiling modes (64-row scores vs 128-row AV) drains the PE and
    serializes LDWEIGHTS at every transition, ~0.5us per j-step.  The
    score matmuls therefore contract the full K=128 against zero-padded
    per-head copies of q (qt_z); the zero half masks the other head's k
    rows at no stream cost (matmul time is N-paced).
  - Projections (v for all 4 heads + q/k both m-tiles) are emitted
    up-front, streamed per 512-column chunk of xT as the DMA lands.
    xT stays resident in SBUF for the whole kernel (bf16 fits easily).
  - Attention runs per (i-block, head-pair): exp split ~75% ACT / ~25%
    DVE (2-pass Schraudolph), AV accumulates o_aug [65, IB] (ones
    column = exp colsum) over the j-loop, lagging scores by DEPTH
    steps.  The per-step pace is set by the exp engines draining the
    two scores PSUM buffers (all 8 PSUM banks are committed).
  - Normalization writes into a 2-head-STACKED o_sb2 [128, kc, ib, i]
    (kc = head pair): even head -> partitions 0-63 directly on the DVE,
    odd head -> partitions 64-127 via a small SBUF->SBUF DMA.  The
    output projection then contracts K=128 per matmul (2 kc chunks
    accumulated in PSUM) -- half the matmuls of the K=64 form and no
    SBUF accumulator / add pass.
  - Output partials are written bf16 (host sums in fp32).
  - A short warm-up burst of dummy matmuls covers the initial DMA wait
    so the PE HAM clock gate is at 8/8 when real work starts.
"""

import struct

import numpy as np
from contextlib import ExitStack

import concourse.bacc as bacc
import concourse.bass as bass
import concourse.mybir as mybir
import concourse.tile as tile
from concourse.bass_utils import run_bass_kernel_spmd

f32 = mybir.dt.float32
bf16 = mybir.dt.bfloat16
i32 = mybir.dt.int32
EXP = mybir.ActivationFunctionType.Exp

# ---- Schraudolph exp on the DVE --------------------------------------------
# pass1 (tensor_scalar): u = int32(score * A + B)
#   A = 0.125*log2(e)*2^23, B = 127*2^23
#   => bitcast(u) = S = 2^i*(1+f) with i+f = score*0.125*log2(e)
# pass2 (fused custom op): r = bitcast((u | 0x3F800000) & 0x3FFFFFFF) = 1+f
#   out = S * (q0 + r*(q1 + r*q2)) ~= S * 2^f/(1+f) = exp(score/8)
EXP_A = float(0.125 * np.log2(np.e) * 2**23)
EXP_B = float(127 * 2**23)
EXP_Q0 = 1.43400066
EXP_Q1 = -0.66623009
EXP_Q2 = 0.22566318
MASK_F = struct.unpack("<f", struct.pack("<I", 0x3FFFFFFF))[0]

_EXP_FUSED = None


def _ensure_exp_fused():
    global _EXP_FUSED
    if _EXP_FUSED is not None:
        return _EXP_FUSED
    import concourse.dve_ops as dve_ops
    from concourse.dve_spec import (
        Spec,
        Src0,
        C0,
        C1,
        C2,
        C3,
        One,
        Bin,
        AluOp,
        _spill_c3_to_src1,
    )

    def _ref(in0, in1, c0, c1, c2):
        u = np.ascontiguousarray(np.asarray(in0, np.float32)).view(np.uint32)
        rb = (u | np.uint32(0x3F800000)) & np.uint32(0x3FFFFFFF)
        r = rb.view(np.float32)
        q0 = np.asarray(in1, np.float32)
        return np.asarray(in0, np.float32) * (q0 + r * (c1 + r * c2))

    r = Bin(AluOp.BITWISE_AND, Bin(AluOp.BITWISE_OR, Src0, One), C0)
    body = Src0 * (C3 + r * (C1 + r * C2))
    op = dve_ops.DveOp(
        "EXP_SFUSE_ANT",
        Spec(body=_spill_c3_to_src1(body), reference=_ref),
        subdim=False,
        uops_sha={},
    )
    if op.name not in dve_ops._SUB_OPCODE_FOR_NAME:
        dve_ops.OPS.append(op)
        dve_ops.CUSTOM_DVE_SPECS[op.name] = op.spec
        dve_ops._SUB_OPCODE_FOR_NAME[op.name] = (
            max(dve_ops._SUB_OPCODE_FOR_NAME.values()) + 1
        )
    for ver in ("v3",):
        try:
            op.compile(ver)
        except ValueError as e:
            msg = str(e)
            got = msg.split(f"{ver}: ")[1].split(" ")[0]
            op.uops_sha[ver] = got
            op.compile(ver)
    _EXP_FUSED = op
    return op


B, S, D = 2, 2048, 1024
H, DH = 16, 64
NCORES = 8
GROUPS = NCORES // B           # 4 head-groups per batch
HPC = H // GROUPS              # 4 heads per core
DLOC = HPC * DH                # 256
KT = D // 128                  # 8 contraction tiles
ST = S // 128                  # 16 sequence tiles
NB = 2                         # i-blocks
IB = S // NB                   # 1024
NCH = IB // 512                # 512-wide matmul chunks per i-block
NSC = 4                        # xT load chunks (columns of 512)
DEPTH = 2                      # j-steps the AV pair lags the scores pair


def _dve_sel(hi, jt):
    """Route ~1/3 of exp tiles to the DVE (2-op Schraudolph), rest ACT.

    Steps with both tiles on ACT pay a ~3.0us serial chain (two 1.07us
    activations back to back) vs ~2.2us for mixed steps, so spread DVE
    tiles to maximize mixed steps -- but keep the first two steps of
    each pair ACT-only so the previous pair's norm work (o_cp/recip/mul
    on the DVE) drains without delaying pass1."""
    return (2 <= jt <= 13 and (2 * jt + hi) % 8 in (2, 6)) or (
        hi == 1 and jt in (6, 10))


def _emit(tc, nc):
    xT = nc.dram_tensor("xT", [D, S], bf16, kind="ExternalInput").ap()
    wqT = nc.dram_tensor("wqT", [D, DLOC], bf16, kind="ExternalInput").ap()
    wkT = nc.dram_tensor("wkT", [D, DLOC], bf16, kind="ExternalInput").ap()
    wvT = nc.dram_tensor("wvT", [D, DLOC], bf16, kind="ExternalInput").ap()
    woT = nc.dram_tensor("woT", [DLOC, D], bf16, kind="ExternalInput").ap()
    outp = nc.dram_tensor("outp", [S, D], bf16, kind="ExternalOutput").ap()

    exp_op = _ensure_exp_fused()
    alu = bass.mybir.AluOpType

    with ExitStack() as ctx:
        wpool = ctx.enter_context(tc.tile_pool(name="wpool", bufs=1))
        qkv = ctx.enter_context(tc.tile_pool(name="qkv", bufs=1))
        small = ctx.enter_context(tc.tile_pool(name="smalls", bufs=2))
        ps = ctx.enter_context(tc.tile_pool(name="ps", bufs=2, space="PSUM"))
        pso = ctx.enter_context(tc.tile_pool(name="pso", bufs=2, space="PSUM"))
        ptp = ctx.enter_context(tc.tile_pool(name="ptp", bufs=12))
        osb = ctx.enter_context(tc.tile_pool(name="osb", bufs=1))
        norm = ctx.enter_context(tc.tile_pool(name="norm", bufs=2))
        outsb = ctx.enter_context(tc.tile_pool(name="outsb", bufs=3))

        # o_sb2: 2-head-stacked normalized attention output.
        # partition p in [0,128): kc chunk holds local dims kc*128+p,
        # i.e. kc=0 -> heads 0,1 and kc=1 -> heads 2,3.
        o_sb2 = osb.tile([128, 2, NB, IB], bf16, name="o_sb2")

        # ---- constants ----
        # Every matmul in this kernel runs in the full (128,128) array mode
        # (K>=65 so row tiling never engages, M>=65 so column tiling never
        # does).  Mode switches drain the PE array and serialize LDWEIGHTS,
        # costing ~0.5us per switch.
        ones_f = small.tile([128, HPC], f32, bufs=1)
        nc.vector.memset(ones_f, 1.0)
        # e64 [65,128]: selector weights, row 64 = 1 -- broadcast matmul
        # lhsT (out[m,n] = rhs[64,n] for all m) in full array mode.
        e64 = small.tile([65, 128], bf16, bufs=1)
        nc.vector.memset(e64, 0.0)
        nc.vector.memset(e64[64:65, :], 1.0)
        # warm weights: K=128, M=65
        ones128 = small.tile([128, 65], bf16, bufs=1)
        nc.vector.memset(ones128, 1.0)

        q0t = small.tile([128, 1], f32, bufs=1)
        nc.vector.memset(q0t, EXP_Q0)

        # ---- HAM warm-keeper ----
        warm_rhs = small.tile([128, 512], bf16, bufs=1)
        nc.vector.memset(warm_rhs, 0.0)

        def warm_burst(k, pool, tag):
            wt = pool.tile([65, 512], f32, tag=tag, name="warm")
            for _ in range(k):
                nc.tensor.matmul(wt, lhsT=ones128, rhs=warm_rhs,
                                 start=True, stop=True)

        warm_burst(14, pso, "pso")

        # ---- weight + xT loads (all bf16, few big DMAs -- trigger cost
        # ~600ns each dominates small transfers) ----
        wts = {}
        for name, src in (("wv", wvT), ("wq", wqT), ("wk", wkT)):
            w_r = wpool.tile([128, KT, DLOC], bf16, name=f"{name}_r", tag=name)
            srcv = src.rearrange("(k p) m -> p k m", p=128)
            nc.gpsimd.dma_start(out=w_r, in_=srcv)
            wts[name] = w_r
        wv_r, wq_r, wk_r = wts["wv"], wts["wq"], wts["wk"]

        # wo2 [128, kc, D]: partition p of chunk kc = local out dim kc*128+p
        wo2 = wpool.tile([128, 2, D], bf16, name="wo2")
        wov = woT.rearrange("(kc p) e -> p kc e", p=128)
        nc.gpsimd.dma_start(out=wo2, in_=wov)

        xt_r = wpool.tile([128, KT, S], bf16, name="xt_r")
        xv = xT.rearrange("(k p) s -> p k s", p=128)
        # sc0 split across both queues so the first projection chunk lands
        # as early as possible (the proj phase is gated on it)
        nc.sync.dma_start(out=xt_r[:, 0:4, 0:512], in_=xv[:, 0:4, 0:512])
        nc.scalar.dma_start(out=xt_r[:, 4:8, 0:512], in_=xv[:, 4:8, 0:512])
        for sc in range(1, NSC):
            lo, hi = sc * (S // NSC), (sc + 1) * (S // NSC)
            eng = nc.sync if sc % 2 == 0 else nc.scalar
            eng.dma_start(out=xt_r[:, :, lo:hi], in_=xv[:, :, lo:hi])

        # ---- projections: v all heads + q/k both m-tiles, streamed per
        # 512-column chunk of xT ----
        # qt_z [128, m, zi, S]: zero-padded per-head q so the score matmuls
        # contract the FULL 128 partitions (kt carries both heads' k; the
        # zero half of q masks the other head).  Keeps every score matmul
        # in (128,128) array mode -- no row-tiling mode switches.
        v_sb = qkv.tile([128, ST, HPC, 65], bf16)
        qt_z = qkv.tile([128, 2, 2, S], bf16)
        kt = qkv.tile([128, 2, S], bf16)
        nc.gpsimd.memset(qt_z[64:128, :, 0, :], 0.0)
        nc.gpsimd.memset(qt_z[0:64, :, 1, :], 0.0)

        def emit_v(st_i):
            pv = ps.tile([128, DLOC], f32, tag="ps", name="pv")
            for k in range(KT):
                nc.tensor.matmul(
                    pv,
                    lhsT=xt_r[:, k, st_i * 128 : (st_i + 1) * 128],
                    rhs=wv_r[:, k],
                    start=(k == 0),
                    stop=(k == KT - 1),
                )
            nc.vector.tensor_copy(
                v_sb[:, st_i, :, 0:64], pv.rearrange("p (h d) -> p h d", h=HPC)
            )
            nc.vector.tensor_copy(v_sb[:, st_i, :, 64], ones_f)

        def emit_qk(dst, w_r, m, sc, ceng, split=False):
            lo = sc * 512
            pq = ps.tile([128, 512], f32, tag="ps", name="pq")
            for k in range(KT):
                nc.tensor.matmul(
                    pq,
                    lhsT=w_r[:, k, m * 128 : (m + 1) * 128],
                    rhs=xt_r[:, k, lo : lo + 512],
                    start=(k == 0),
                    stop=(k == KT - 1),
                )
            if split:
                # even head dims -> zi=0 rows 0-63, odd -> zi=1 rows 64-127
                if ceng is nc.vector:
                    ceng.tensor_copy(dst[0:64, m, 0, lo : lo + 512], pq[0:64])
                    ceng.tensor_copy(dst[64:128, m, 1, lo : lo + 512],
                                     pq[64:128])
                else:
                    ceng.copy(dst[0:64, m, 0, lo : lo + 512], pq[0:64])
                    ceng.copy(dst[64:128, m, 1, lo : lo + 512], pq[64:128])
            elif ceng is nc.vector:
                ceng.tensor_copy(dst[:, m, lo : lo + 512], pq)
            else:
                ceng.copy(dst[:, m, lo : lo + 512], pq)

        def proj_chunk(sc):
            for st_i in range(4 * sc, 4 * sc + 4):
                emit_v(st_i)
            emit_qk(qt_z, wq_r, 0, sc, nc.scalar, split=True)
            emit_qk(kt, wk_r, 0, sc, nc.vector)
            emit_qk(qt_z, wq_r, 1, sc, nc.scalar, split=True)
            emit_qk(kt, wk_r, 1, sc, nc.vector)

        def pair_stepper(ib, h0, extra=None, norm_chunks=1):
            """Attention for heads (h0, h0+1) over i-block ib, as a
            generator yielding after each j-step so the caller can
            interleave other emission (projection chunks).  AV lags by
            DEPTH j-steps; `extra` PE filler closures pop on late steps."""
            heads = (h0, h0 + 1)
            kc = h0 // 2
            o_augs = {
                h: pso.tile([65, IB], f32, tag="pso", name="o_aug")
                for h in heads
            }

            def scores_pair(jt):
                sscs = {}
                for h in heads:
                    sscs[h] = ps.tile([128, IB], f32, tag="ps", name="ssc")
                for ch in range(NCH):
                    for h in heads:
                        zi = h % 2
                        mi = h // 2
                        nc.tensor.matmul(
                            sscs[h][:, ch * 512 : (ch + 1) * 512],
                            lhsT=kt[:, mi, jt * 128 : (jt + 1) * 128],
                            rhs=qt_z[
                                :,
                                mi,
                                zi,
                                ib * IB + ch * 512 : ib * IB + (ch + 1) * 512,
                            ],
                            start=True,
                            stop=True,
                        )
                pts = {}
                for h in heads:
                    pt = ptp.tile([128, IB], bf16, tag="pt", name="pt")
                    if _dve_sel(h - h0, jt):
                        ue = ptp.tile([128, IB], i32, tag="ue", name="ue",
                                      bufs=2)
                        nc.vector.tensor_scalar(
                            ue, sscs[h], EXP_A, EXP_B, alu.mult, alu.add
                        )
                        nc.vector._custom_dve(
                            exp_op,
                            out=pt,
                            in0=ue.bitcast(f32),
                            in1=q0t,
                            s0=MASK_F,
                            s1=EXP_Q1,
                            imm2=EXP_Q2,
                        )
                    else:
                        nc.scalar.activation(pt, sscs[h], EXP, scale=0.125)
                    pts[h] = (pt, None)
                return pts

            def av_pair(jt, pts):
                for ch in range(NCH):
                    for h in heads:
                        nc.tensor.matmul(
                            o_augs[h][:, ch * 512 : (ch + 1) * 512],
                            lhsT=v_sb[:, jt, h, :],
                            rhs=pts[h][0][:, ch * 512 : (ch + 1) * 512],
                            start=(jt == 0),
                            stop=(jt == ST - 1),
                        )

            # filler pops start late in the j-loop so the previous pair's
            # norm chain (o_cp -> cb -> recip -> mul -> DMA) has completed
            # before a filler that depends on it enters the in-order PE queue
            ex = list(extra or [])
            n_ex = len(ex)
            pops = set()
            if n_ex:
                lo_n = ST - 2 * n_ex
                pops = {lo_n + 2 * i + 1 for i in range(n_ex)}
            pend = {}
            for n in range(ST):
                pend[n] = scores_pair(n)
                if ex and n in pops:
                    ex.pop(0)()
                if n >= DEPTH:
                    av_pair(n - DEPTH, pend.pop(n - DEPTH))
                yield
            for n in range(ST - DEPTH, ST):
                av_pair(n, pend.pop(n))
            for fn in ex:
                fn()

            # normalization into the stacked o_sb2:
            #   even head -> partitions 0-63 (direct DVE write)
            #   odd head  -> partitions 64-127 (via SBUF->SBUF DMA)
            # norm_chunks=2 processes 512-column halves with a yield in
            # between, so the caller can start output projections on the
            # first half while the second half's chain is still running
            # (used for the final pair, whose norm latency is exposed).
            if norm_chunks == 1:
                for h in heads:
                    o_cp = norm.tile([65, IB], bf16, tag="ocp", name="o_cp")
                    if h == heads[0]:
                        nc.scalar.copy(o_cp, o_augs[h])
                    else:
                        nc.vector.tensor_copy(o_cp, o_augs[h])
                    # broadcast row 64 (exp colsum) to all partitions via
                    # e64 selector weights -- K=65, full (128,128) mode
                    cb_ps = pso.tile([128, IB], f32, tag="pso", name="cb_ps")
                    for ch in range(NCH):
                        nc.tensor.matmul(
                            cb_ps[:, ch * 512 : (ch + 1) * 512],
                            lhsT=e64,
                            rhs=o_cp[:, ch * 512 : (ch + 1) * 512],
                            start=True,
                            stop=True,
                        )
                    rb_f = norm.tile([64, IB], f32, tag="rb", name="rb_f")
                    nc.vector.reciprocal_approx_fast(rb_f, cb_ps[0:64, :])
                    if h % 2 == 0:
                        nc.vector.tensor_mul(
                            o_sb2[0:64, kc, ib], o_cp[0:64, :], rb_f
                        )
                    else:
                        nm = norm.tile([64, IB], bf16, tag="nm", name="nm")
                        nc.vector.tensor_mul(nm, o_cp[0:64, :], rb_f)
                        # gpsimd queue: idle at norm time, so the trigger
                        # fires as soon as the mul's semaphore lands
                        nc.gpsimd.dma_start(out=o_sb2[64:128, kc, ib],
                                            in_=nm)
            else:
                # copy both heads fully first (releases the o_aug PSUM
                # buffers so the cb_ps allocations below can't deadlock
                # against the 2-buffer pso pool)
                o_cps = {}
                for h in heads:
                    o_cp = norm.tile([65, IB], bf16, tag="ocp", name="o_cp")
                    eng = nc.scalar if h == heads[0] else None
                    for c in range(2):
                        cs = slice(c * 512, (c + 1) * 512)
                        if eng is nc.scalar:
                            nc.scalar.copy(o_cp[:, cs], o_augs[h][:, cs])
                        else:
                            nc.vector.tensor_copy(o_cp[:, cs],
                                                  o_augs[h][:, cs])
                    o_cps[h] = o_cp
                for c in range(2):
                    cs = slice(c * 512, (c + 1) * 512)
                    for h in heads:
                        cb_ps = pso.tile([128, 512], f32, tag="pso",
                                         name="cb_ps")
                        nc.tensor.matmul(cb_ps, lhsT=e64,
                                         rhs=o_cps[h][:, cs],
                                         start=True, stop=True)
                        rb_f = norm.tile([64, 512], f32, tag="rb",
                                         name="rb_f")
                        nc.vector.reciprocal_approx_fast(rb_f,
                                                         cb_ps[0:64, :])
                        if h % 2 == 0:
                            nc.vector.tensor_mul(
                                o_sb2[0:64, kc, ib, cs],
                                o_cps[h][0:64, cs], rb_f
                            )
                        else:
                            nm = norm.tile([64, 512], bf16, tag="nm",
                                           name="nm")
                            nc.vector.tensor_mul(nm, o_cps[h][0:64, cs],
                                                 rb_f)
                            nc.gpsimd.dma_start(
                                out=o_sb2[64:128, kc, ib, cs], in_=nm
                            )
                    if c == 0:
                        yield

        def emit_po(ib, it, dve_copy=False, final=False):
            """Output projection for i-tile it of i-block ib (all 4 heads,
            two K=128 chunks accumulated in PSUM)."""
            po = ps.tile([128, D], f32, tag="ps", name="po")
            for ch in range(2):
                for kc in range(2):
                    nc.tensor.matmul(
                        po[:, ch * 512 : (ch + 1) * 512],
                        lhsT=o_sb2[:, kc, ib, it * 128 : (it + 1) * 128],
                        rhs=wo2[:, kc, ch * 512 : (ch + 1) * 512],
                        start=(kc == 0),
                        stop=(kc == 1),
                    )
            ot = outsb.tile([128, D], bf16, tag="ot", name="ot")
            row = ib * IB + it * 128
            if final:
                # tail-latency critical: halve the copy+DMA chain by
                # splitting across both engines and both DMA queues
                nc.scalar.copy(ot[:, 0:512], po[:, 0:512])
                nc.vector.tensor_copy(ot[:, 512:1024], po[:, 512:1024])
                nc.sync.dma_start(out=outp[row : row + 128, 0:512],
                                  in_=ot[:, 0:512])
                nc.scalar.dma_start(out=outp[row : row + 128, 512:1024],
                                    in_=ot[:, 512:1024])
                return
            if dve_copy or it % 2 == 1:
                nc.vector.tensor_copy(ot, po)
            else:
                nc.scalar.copy(ot, po)
            eng = nc.sync if it % 2 == 0 else nc.scalar
            eng.dma_start(out=outp[row : row + 128, :], in_=ot)

        # ---- attention pairs; output projection of ib0 fills pair (1,0)
        # and pair (1,2) ----
        def emit_head_pair(ib, h0, extra=None):
            for _ in pair_stepper(ib, h0, extra):
                pass

        # ---- projections interleaved with pair (0,0)'s j-loop: j-step jt
        # needs kt/v_sb columns from chunk sc = jt//4, and the ib=0 q
        # columns from chunks 0-1.  The exp engines are otherwise idle for
        # the whole projection phase; riding pair (0,0) under it removes
        # one exp-paced pair from the attention span.
        g0 = pair_stepper(0, 0)
        proj_chunk(0)
        proj_chunk(1)
        for _ in range(8):
            next(g0)
        proj_chunk(2)
        for _ in range(4):
            next(g0)
        proj_chunk(3)
        for _ in g0:
            pass

        emit_head_pair(0, 2)
        emit_head_pair(1, 0, extra=[lambda it=it: emit_po(0, it)
                                    for it in range(4)])
        g3 = pair_stepper(1, 2, extra=[lambda it=it: emit_po(0, it)
                                       for it in range(4, 8)],
                          norm_chunks=2)
        for _ in range(ST):
            next(g3)
        next(g3)  # AV drain + fillers + norm first half
        for it in range(4):
            emit_po(1, it)
        for _ in g3:  # norm second half
            pass
        for it in range(4, 6):
            emit_po(1, it)
        for it in range(6, 8):
            emit_po(1, it, final=True)


_PROGRAM = None


def _program():
    global _PROGRAM
    if _PROGRAM is None:
        nc = bacc.Bacc("TRN2", target_bir_lowering=False, debug=False)
        with tile.TileContext(nc) as tc:
            _emit(tc, nc)
        nc.compile()
        _PROGRAM = nc
    return _PROGRAM


def make_in_maps(x, wq, wk, wv, wo):
    """Per-core bf16 input maps (shared by kernel() and test harness)."""
    import ml_dtypes

    bf = ml_dtypes.bfloat16
    x = np.asarray(x, np.float32)
    wq = np.asarray(wq, np.float32)
    wk = np.asarray(wk, np.float32)
    wv = np.asarray(wv, np.float32)
    wo = np.asarray(wo, np.float32)
    in_maps = []
    for c in range(NCORES):
        b, g = divmod(c, GROUPS)
        rows = slice(g * DLOC, (g + 1) * DLOC)
        in_maps.append(
            {
                "xT": np.ascontiguousarray(x[b].T).astype(bf),
                "wqT": np.ascontiguousarray(wq[rows, :].T).astype(bf),
                "wkT": np.ascontiguousarray(wk[rows, :].T).astype(bf),
                "wvT": np.ascontiguousarray(wv[rows, :].T).astype(bf),
                "woT": np.ascontiguousarray(wo[:, rows].T).astype(bf),
            }
        )
    return in_maps


def kernel(x, e, wq, wk, wv, wo, **_unused):
    nc = _program()
    in_maps = make_in_maps(x, wq, wk, wv, wo)

    # Transient device corruption has been observed on this fabric
    # (NRT_EXEC_UNIT_UNRECOVERABLE events); sanity-check the partials and
    # retry up to twice if a core returned garbage.
    def _sane(parts):
        for p in parts:
            if not np.isfinite(p).all():
                return False
            amax = np.abs(p).max()
            if amax > 1e6 or amax == 0.0:
                return False
            if (np.abs(p).max(axis=1) == 0.0).any():
                return False
        return True

    parts = None
    for _attempt in range(3):
        res = run_bass_kernel_spmd(nc, in_maps, list(range(NCORES))).results
        parts = [np.asarray(res[c]["outp"], dtype=np.float32)
                 for c in range(NCORES)]
        if _sane(parts):
            break

    out = np.empty((B, S, D), dtype=np.float32)
    for b in range(B):
        acc = parts[b * GROUPS]
        for g in range(1, GROUPS):
            acc = acc + parts[b * GROUPS + g]
        out[b] = acc
    return out


# revision 47
# speedup vs baseline: 1.0278x; 1.0278x over previous
"""Multi-head attention forward on 8 Trainium2 NeuronCores.

Problem: x [2,2048,1024], weights wq/wk/wv/wo [1024,1024] (torch Linear
layout, y = x @ W.T), 16 heads, head_dim 64, fp32.

Sharding: core c handles batch b = c//4 and head group g = c%4 (heads
4g..4g+3, i.e. 256 output dims of wq/wk/wv and 256 input dims of wo).
Each core computes a partial output [2048, 1024]; the host sums the 4
partials per batch (the reduce is host-side, no collectives).

On-core plan (v5, 239us vs the 335us v2 baseline):
  - All inputs are host-cast to bf16; DMA traffic halves vs fp32 and
    every matmul runs bf16 (full PE rate + fast weight load).  Few big
    DMA triggers (~600ns each); the first xT chunk is split across the
    sync+scalar queues so the projection phase starts earliest.
  - EVERY matmul runs in the full (128,128) array mode.  Mixing array
    tiling modes (64-row scores vs 128-row AV) drains the PE and
    serializes LDWEIGHTS at every transition, ~0.5us per j-step.  The
    score matmuls therefore contract the full K=128 against zero-padded
    per-head copies of q (qt_z); the zero half masks the other head's k
    rows at no stream cost (matmul time is N-paced).
  - Projections (v for all 4 heads + q/k both m-tiles) are emitted
    up-front, streamed per 512-column chunk of xT as the DMA lands.
    xT stays resident in SBUF for the whole kernel (bf16 fits easily).
  - Attention runs per (i-block, head-pair): exp split ~75% ACT / ~25%
    DVE (2-pass Schraudolph), AV accumulates o_aug [65, IB] (ones
    column = exp colsum) over the j-loop, lagging scores by DEPTH
    steps.  The per-step pace is set by the exp engines draining the
    two scores PSUM buffers (all 8 PSUM banks are committed).
  - Normalization writes into a 2-head-STACKED o_sb2 [128, kc, ib, i]
    (kc = head pair): even head -> partitions 0-63 directly on the DVE,
    odd head -> partitions 64-127 via a small SBUF->SBUF DMA.  The
    output projection then contracts K=128 per matmul (2 kc chunks
    accumulated in PSUM) -- half the matmuls of the K=64 form and no
    SBUF accumulator / add pass.
  - Output partials are written bf16 (host sums in fp32).
  - A short warm-up burst of dummy matmuls covers the initial DMA wait
    so the PE HAM clock gate is at 8/8 when real work starts.
"""

import struct

import numpy as np
from contextlib import ExitStack

import concourse.bacc as bacc
import concourse.bass as bass
import concourse.mybir as mybir
import concourse.tile as tile
from concourse.bass_utils import run_bass_kernel_spmd

f32 = mybir.dt.float32
bf16 = mybir.dt.bfloat16
i32 = mybir.dt.int32
EXP = mybir.ActivationFunctionType.Exp

# ---- Schraudolph exp on the DVE --------------------------------------------
# pass1 (tensor_scalar): u = int32(score * A + B)
#   A = 0.125*log2(e)*2^23, B = 127*2^23
#   => bitcast(u) = S = 2^i*(1+f) with i+f = score*0.125*log2(e)
# pass2 (fused custom op): r = bitcast((u | 0x3F800000) & 0x3FFFFFFF) = 1+f
#   out = S * (q0 + r*(q1 + r*q2)) ~= S * 2^f/(1+f) = exp(score/8)
EXP_A = float(0.125 * np.log2(np.e) * 2**23)
EXP_B = float(127 * 2**23)
EXP_Q0 = 1.43400066
EXP_Q1 = -0.66623009
EXP_Q2 = 0.22566318
MASK_F = struct.unpack("<f", struct.pack("<I", 0x3FFFFFFF))[0]

_EXP_FUSED = None


def _ensure_exp_fused():
    global _EXP_FUSED
    if _EXP_FUSED is not None:
        return _EXP_FUSED
    import concourse.dve_ops as dve_ops
    from concourse.dve_spec import (
        Spec,
        Src0,
        C0,
        C1,
        C2,
        C3,
        One,
        Bin,
        AluOp,
        _spill_c3_to_src1,
    )

    def _ref(in0, in1, c0, c1, c2):
        u = np.ascontiguousarray(np.asarray(in0, np.float32)).view(np.uint32)
        rb = (u | np.uint32(0x3F800000)) & np.uint32(0x3FFFFFFF)
        r = rb.view(np.float32)
        q0 = np.asarray(in1, np.float32)
        return np.asarray(in0, np.float32) * (q0 + r * (c1 + r * c2))

    r = Bin(AluOp.BITWISE_AND, Bin(AluOp.BITWISE_OR, Src0, One), C0)
    body = Src0 * (C3 + r * (C1 + r * C2))
    op = dve_ops.DveOp(
        "EXP_SFUSE_ANT",
        Spec(body=_spill_c3_to_src1(body), reference=_ref),
        subdim=False,
        uops_sha={},
    )
    if op.name not in dve_ops._SUB_OPCODE_FOR_NAME:
        dve_ops.OPS.append(op)
        dve_ops.CUSTOM_DVE_SPECS[op.name] = op.spec
        dve_ops._SUB_OPCODE_FOR_NAME[op.name] = (
            max(dve_ops._SUB_OPCODE_FOR_NAME.values()) + 1
        )
    for ver in ("v3",):
        try:
            op.compile(ver)
        except ValueError as e:
            msg = str(e)
            got = msg.split(f"{ver}: ")[1].split(" ")[0]
            op.uops_sha[ver] = got
            op.compile(ver)
    _EXP_FUSED = op
    return op


B, S, D = 2, 2048, 1024
H, DH = 16, 64
NCORES = 8
GROUPS = NCORES // B           # 4 head-groups per batch
HPC = H // GROUPS              # 4 heads per core
DLOC = HPC * DH                # 256
KT = D // 128                  # 8 contraction tiles
ST = S // 128                  # 16 sequence tiles
NB = 2                         # i-blocks
IB = S // NB                   # 1024
NCH = IB // 512                # 512-wide matmul chunks per i-block
NSC = 4                        # xT load chunks (columns of 512)
DEPTH = 2                      # j-steps the AV pair lags the scores pair


def _dve_sel(hi, jt):
    """Route ~1/3 of exp tiles to the DVE (2-op Schraudolph), rest ACT.

    Steps with both tiles on ACT pay a ~3.0us serial chain (two 1.07us
    activations back to back) vs ~2.2us for mixed steps, so spread DVE
    tiles to maximize mixed steps -- but keep the first two steps of
    each pair ACT-only so the previous pair's norm work (o_cp/recip/mul
    on the DVE) drains without delaying pass1."""
    return 2 <= jt <= 13 and (2 * jt + hi) % 16 in (2, 10)


def _emit(tc, nc):
    xT = nc.dram_tensor("xT", [D, S], bf16, kind="ExternalInput").ap()
    wqT = nc.dram_tensor("wqT", [D, DLOC], bf16, kind="ExternalInput").ap()
    wkT = nc.dram_tensor("wkT", [D, DLOC], bf16, kind="ExternalInput").ap()
    wvT = nc.dram_tensor("wvT", [D, DLOC], bf16, kind="ExternalInput").ap()
    woT = nc.dram_tensor("woT", [DLOC, D], bf16, kind="ExternalInput").ap()
    outp = nc.dram_tensor("outp", [S, D], bf16, kind="ExternalOutput").ap()

    exp_op = _ensure_exp_fused()
    alu = bass.mybir.AluOpType

    with ExitStack() as ctx:
        wpool = ctx.enter_context(tc.tile_pool(name="wpool", bufs=1))
        qkv = ctx.enter_context(tc.tile_pool(name="qkv", bufs=1))
        small = ctx.enter_context(tc.tile_pool(name="smalls", bufs=2))
        ps = ctx.enter_context(tc.tile_pool(name="ps", bufs=2, space="PSUM"))
        pso = ctx.enter_context(tc.tile_pool(name="pso", bufs=2, space="PSUM"))
        ptp = ctx.enter_context(tc.tile_pool(name="ptp", bufs=12))
        osb = ctx.enter_context(tc.tile_pool(name="osb", bufs=1))
        norm = ctx.enter_context(tc.tile_pool(name="norm", bufs=2))
        outsb = ctx.enter_context(tc.tile_pool(name="outsb", bufs=3))

        # o_sb2: 2-head-stacked normalized attention output.
        # partition p in [0,128): kc chunk holds local dims kc*128+p,
        # i.e. kc=0 -> heads 0,1 and kc=1 -> heads 2,3.
        o_sb2 = osb.tile([128, 2, NB, IB], bf16, name="o_sb2")

        # ---- constants ----
        # Every matmul in this kernel runs in the full (128,128) array mode
        # (K>=65 so row tiling never engages, M>=65 so column tiling never
        # does).  Mode switches drain the PE array and serialize LDWEIGHTS,
        # costing ~0.5us per switch.
        ones_f = small.tile([128, HPC], f32, bufs=1)
        nc.vector.memset(ones_f, 1.0)
        # e64 [65,128]: selector weights, row 64 = 1 -- broadcast matmul
        # lhsT (out[m,n] = rhs[64,n] for all m) in full array mode.
        e64 = small.tile([65, 128], bf16, bufs=1)
        nc.vector.memset(e64, 0.0)
        nc.vector.memset(e64[64:65, :], 1.0)
        # warm weights: K=128, M=65
        ones128 = small.tile([128, 65], bf16, bufs=1)
        nc.vector.memset(ones128, 1.0)

        q0t = small.tile([128, 1], f32, bufs=1)
        nc.vector.memset(q0t, EXP_Q0)

        # ---- HAM warm-keeper ----
        warm_rhs = small.tile([128, 512], bf16, bufs=1)
        nc.vector.memset(warm_rhs, 0.0)

        def warm_burst(k, pool, tag):
            wt = pool.tile([65, 512], f32, tag=tag, name="warm")
            for _ in range(k):
                nc.tensor.matmul(wt, lhsT=ones128, rhs=warm_rhs,
                                 start=True, stop=True)

        warm_burst(14, pso, "pso")

        # ---- weight + xT loads (all bf16, few big DMAs -- trigger cost
        # ~600ns each dominates small transfers) ----
        wts = {}
        for name, src in (("wv", wvT), ("wq", wqT), ("wk", wkT)):
            w_r = wpool.tile([128, KT, DLOC], bf16, name=f"{name}_r", tag=name)
            srcv = src.rearrange("(k p) m -> p k m", p=128)
            nc.gpsimd.dma_start(out=w_r, in_=srcv)
            wts[name] = w_r
        wv_r, wq_r, wk_r = wts["wv"], wts["wq"], wts["wk"]

        # wo2 [128, kc, D]: partition p of chunk kc = local out dim kc*128+p
        wo2 = wpool.tile([128, 2, D], bf16, name="wo2")
        wov = woT.rearrange("(kc p) e -> p kc e", p=128)
        nc.gpsimd.dma_start(out=wo2, in_=wov)

        xt_r = wpool.tile([128, KT, S], bf16, name="xt_r")
        xv = xT.rearrange("(k p) s -> p k s", p=128)
        # sc0 split across both queues so the first projection chunk lands
        # as early as possible (the proj phase is gated on it)
        nc.sync.dma_start(out=xt_r[:, 0:4, 0:512], in_=xv[:, 0:4, 0:512])
        nc.scalar.dma_start(out=xt_r[:, 4:8, 0:512], in_=xv[:, 4:8, 0:512])
        for sc in range(1, NSC):
            lo, hi = sc * (S // NSC), (sc + 1) * (S // NSC)
            eng = nc.sync if sc % 2 == 0 else nc.scalar
            eng.dma_start(out=xt_r[:, :, lo:hi], in_=xv[:, :, lo:hi])

        # ---- projections: v all heads + q/k both m-tiles, streamed per
        # 512-column chunk of xT ----
        # qt_z [128, m, zi, S]: zero-padded per-head q so the score matmuls
        # contract the FULL 128 partitions (kt carries both heads' k; the
        # zero half of q masks the other head).  Keeps every score matmul
        # in (128,128) array mode -- no row-tiling mode switches.
        v_sb = qkv.tile([128, ST, HPC, 65], bf16)
        qt_z = qkv.tile([128, 2, 2, S], bf16)
        kt = qkv.tile([128, 2, S], bf16)
        nc.gpsimd.memset(qt_z[64:128, :, 0, :], 0.0)
        nc.gpsimd.memset(qt_z[0:64, :, 1, :], 0.0)

        def emit_v(st_i):
            pv = ps.tile([128, DLOC], f32, tag="ps", name="pv")
            for k in range(KT):
                nc.tensor.matmul(
                    pv,
                    lhsT=xt_r[:, k, st_i * 128 : (st_i + 1) * 128],
                    rhs=wv_r[:, k],
                    start=(k == 0),
                    stop=(k == KT - 1),
                )
            nc.vector.tensor_copy(
                v_sb[:, st_i, :, 0:64], pv.rearrange("p (h d) -> p h d", h=HPC)
            )
            nc.vector.tensor_copy(v_sb[:, st_i, :, 64], ones_f)

        def emit_qk(dst, w_r, m, sc, ceng, split=False):
            lo = sc * 512
            pq = ps.tile([128, 512], f32, tag="ps", name="pq")
            for k in range(KT):
                nc.tensor.matmul(
                    pq,
                    lhsT=w_r[:, k, m * 128 : (m + 1) * 128],
                    rhs=xt_r[:, k, lo : lo + 512],
                    start=(k == 0),
                    stop=(k == KT - 1),
                )
            if split:
                # even head dims -> zi=0 rows 0-63, odd -> zi=1 rows 64-127
                if ceng is nc.vector:
                    ceng.tensor_copy(dst[0:64, m, 0, lo : lo + 512], pq[0:64])
                    ceng.tensor_copy(dst[64:128, m, 1, lo : lo + 512],
                                     pq[64:128])
                else:
                    ceng.copy(dst[0:64, m, 0, lo : lo + 512], pq[0:64])
                    ceng.copy(dst[64:128, m, 1, lo : lo + 512], pq[64:128])
            elif ceng is nc.vector:
                ceng.tensor_copy(dst[:, m, lo : lo + 512], pq)
            else:
                ceng.copy(dst[:, m, lo : lo + 512], pq)

        def proj_chunk(sc):
            for st_i in range(4 * sc, 4 * sc + 4):
                emit_v(st_i)
            emit_qk(qt_z, wq_r, 0, sc, nc.scalar, split=True)
            emit_qk(kt, wk_r, 0, sc, nc.vector)
            emit_qk(qt_z, wq_r, 1, sc, nc.scalar, split=True)
            emit_qk(kt, wk_r, 1, sc, nc.vector)

        def pair_stepper(ib, h0, extra=None, norm_chunks=1):
            """Attention for heads (h0, h0+1) over i-block ib, as a
            generator yielding after each j-step so the caller can
            interleave other emission (projection chunks).  AV lags by
            DEPTH j-steps; `extra` PE filler closures pop on late steps."""
            heads = (h0, h0 + 1)
            kc = h0 // 2
            o_augs = {
                h: pso.tile([65, IB], f32, tag="pso", name="o_aug")
                for h in heads
            }

            def scores_pair(jt):
                sscs = {}
                for h in heads:
                    sscs[h] = ps.tile([128, IB], f32, tag="ps", name="ssc")
                for ch in range(NCH):
                    for h in heads:
                        zi = h % 2
                        mi = h // 2
                        nc.tensor.matmul(
                            sscs[h][:, ch * 512 : (ch + 1) * 512],
                            lhsT=kt[:, mi, jt * 128 : (jt + 1) * 128],
                            rhs=qt_z[
                                :,
                                mi,
                                zi,
                                ib * IB + ch * 512 : ib * IB + (ch + 1) * 512,
                            ],
                            start=True,
                            stop=True,
                        )
                pts = {}
                for h in heads:
                    pt = ptp.tile([128, IB], bf16, tag="pt", name="pt")
                    if _dve_sel(h - h0, jt):
                        ue = ptp.tile([128, IB], i32, tag="ue", name="ue",
                                      bufs=2)
                        nc.vector.tensor_scalar(
                            ue, sscs[h], EXP_A, EXP_B, alu.mult, alu.add
                        )
                        nc.vector._custom_dve(
                            exp_op,
                            out=pt,
                            in0=ue.bitcast(f32),
                            in1=q0t,
                            s0=MASK_F,
                            s1=EXP_Q1,
                            imm2=EXP_Q2,
                        )
                    else:
                        nc.scalar.activation(pt, sscs[h], EXP, scale=0.125)
                    pts[h] = (pt, None)
                return pts

            def av_pair(jt, pts):
                for ch in range(NCH):
                    for h in heads:
                        nc.tensor.matmul(
                            o_augs[h][:, ch * 512 : (ch + 1) * 512],
                            lhsT=v_sb[:, jt, h, :],
                            rhs=pts[h][0][:, ch * 512 : (ch + 1) * 512],
                            start=(jt == 0),
                            stop=(jt == ST - 1),
                        )

            # filler pops start late in the j-loop so the previous pair's
            # norm chain (o_cp -> cb -> recip -> mul -> DMA) has completed
            # before a filler that depends on it enters the in-order PE queue
            ex = list(extra or [])
            n_ex = len(ex)
            pops = set()
            if n_ex:
                lo_n = ST - 2 * n_ex
                pops = {lo_n + 2 * i + 1 for i in range(n_ex)}
            pend = {}
            for n in range(ST):
                pend[n] = scores_pair(n)
                if ex and n in pops:
                    ex.pop(0)()
                if n >= DEPTH:
                    av_pair(n - DEPTH, pend.pop(n - DEPTH))
                yield
            for n in range(ST - DEPTH, ST):
                av_pair(n, pend.pop(n))
            for fn in ex:
                fn()

            # normalization into the stacked o_sb2:
            #   even head -> partitions 0-63 (direct DVE write)
            #   odd head  -> partitions 64-127 (via SBUF->SBUF DMA)
            # norm_chunks=2 processes 512-column halves with a yield in
            # between, so the caller can start output projections on the
            # first half while the second half's chain is still running
            # (used for the final pair, whose norm latency is exposed).
            if norm_chunks == 1:
                for h in heads:
                    o_cp = norm.tile([65, IB], bf16, tag="ocp", name="o_cp")
                    if h == heads[0]:
                        nc.scalar.copy(o_cp, o_augs[h])
                    else:
                        nc.vector.tensor_copy(o_cp, o_augs[h])
                    # broadcast row 64 (exp colsum) to all partitions via
                    # e64 selector weights -- K=65, full (128,128) mode
                    cb_ps = pso.tile([128, IB], f32, tag="pso", name="cb_ps")
                    for ch in range(NCH):
                        nc.tensor.matmul(
                            cb_ps[:, ch * 512 : (ch + 1) * 512],
                            lhsT=e64,
                            rhs=o_cp[:, ch * 512 : (ch + 1) * 512],
                            start=True,
                            stop=True,
                        )
                    rb_f = norm.tile([64, IB], f32, tag="rb", name="rb_f")
                    nc.vector.reciprocal_approx_fast(rb_f, cb_ps[0:64, :])
                    if h % 2 == 0:
                        nc.vector.tensor_mul(
                            o_sb2[0:64, kc, ib], o_cp[0:64, :], rb_f
                        )
                    else:
                        nm = norm.tile([64, IB], bf16, tag="nm", name="nm")
                        nc.vector.tensor_mul(nm, o_cp[0:64, :], rb_f)
                        # gpsimd queue: idle at norm time, so the trigger
                        # fires as soon as the mul's semaphore lands
                        nc.gpsimd.dma_start(out=o_sb2[64:128, kc, ib],
                                            in_=nm)
            else:
                # copy both heads fully first (releases the o_aug PSUM
                # buffers so the cb_ps allocations below can't deadlock
                # against the 2-buffer pso pool)
                o_cps = {}
                for h in heads:
                    o_cp = norm.tile([65, IB], bf16, tag="ocp", name="o_cp")
                    eng = nc.scalar if h == heads[0] else None
                    for c in range(2):
                        cs = slice(c * 512, (c + 1) * 512)
                        if eng is nc.scalar:
                            nc.scalar.copy(o_cp[:, cs], o_augs[h][:, cs])
                        else:
                            nc.vector.tensor_copy(o_cp[:, cs],
                                                  o_augs[h][:, cs])
                    o_cps[h] = o_cp
                for c in range(2):
                    cs = slice(c * 512, (c + 1) * 512)
                    for h in heads:
                        cb_ps = pso.tile([128, 512], f32, tag="pso",
                                         name="cb_ps")
                        nc.tensor.matmul(cb_ps, lhsT=e64,
                                         rhs=o_cps[h][:, cs],
                                         start=True, stop=True)
                        rb_f = norm.tile([64, 512], f32, tag="rb",
                                         name="rb_f")
                        nc.vector.reciprocal_approx_fast(rb_f,
                                                         cb_ps[0:64, :])
                        if h % 2 == 0:
                            nc.vector.tensor_mul(
                                o_sb2[0:64, kc, ib, cs],
                                o_cps[h][0:64, cs], rb_f
                            )
                        else:
                            nm = norm.tile([64, 512], bf16, tag="nm",
                                           name="nm")
                            nc.vector.tensor_mul(nm, o_cps[h][0:64, cs],
                                                 rb_f)
                            nc.gpsimd.dma_start(
                                out=o_sb2[64:128, kc, ib, cs], in_=nm
                            )
                    if c == 0:
                        yield

        def emit_po(ib, it, dve_copy=False, final=False):
            """Output projection for i-tile it of i-block ib (all 4 heads,
            two K=128 chunks accumulated in PSUM)."""
            po = ps.tile([128, D], f32, tag="ps", name="po")
            for ch in range(2):
                for kc in range(2):
                    nc.tensor.matmul(
                        po[:, ch * 512 : (ch + 1) * 512],
                        lhsT=o_sb2[:, kc, ib, it * 128 : (it + 1) * 128],
                        rhs=wo2[:, kc, ch * 512 : (ch + 1) * 512],
                        start=(kc == 0),
                        stop=(kc == 1),
                    )
            ot = outsb.tile([128, D], bf16, tag="ot", name="ot")
            row = ib * IB + it * 128
            if final:
                # tail-latency critical: halve the copy+DMA chain by
                # splitting across both engines and both DMA queues
                nc.scalar.copy(ot[:, 0:512], po[:, 0:512])
                nc.vector.tensor_copy(ot[:, 512:1024], po[:, 512:1024])
                nc.sync.dma_start(out=outp[row : row + 128, 0:512],
                                  in_=ot[:, 0:512])
                nc.scalar.dma_start(out=outp[row : row + 128, 512:1024],
                                    in_=ot[:, 512:1024])
                return
            if dve_copy or it % 2 == 1:
                nc.vector.tensor_copy(ot, po)
            else:
                nc.scalar.copy(ot, po)
            eng = nc.sync if it % 2 == 0 else nc.scalar
            eng.dma_start(out=outp[row : row + 128, :], in_=ot)

        # ---- attention pairs; output projection of ib0 fills pair (1,0)
        # and pair (1,2) ----
        def emit_head_pair(ib, h0, extra=None):
            for _ in pair_stepper(ib, h0, extra):
                pass

        # ---- projections interleaved with pair (0,0)'s j-loop: j-step jt
        # needs kt/v_sb columns from chunk sc = jt//4, and the ib=0 q
        # columns from chunks 0-1.  The exp engines are otherwise idle for
        # the whole projection phase; riding pair (0,0) under it removes
        # one exp-paced pair from the attention span.
        g0 = pair_stepper(0, 0)
        proj_chunk(0)
        proj_chunk(1)
        for _ in range(8):
            next(g0)
        proj_chunk(2)
        for _ in range(4):
            next(g0)
        proj_chunk(3)
        for _ in g0:
            pass

        emit_head_pair(0, 2)
        emit_head_pair(1, 0, extra=[lambda it=it: emit_po(0, it)
                                    for it in range(4)])
        g3 = pair_stepper(1, 2, extra=[lambda it=it: emit_po(0, it)
                                       for it in range(4, 8)],
                          norm_chunks=2)
        for _ in range(ST):
            next(g3)
        next(g3)  # AV drain + fillers + norm first half
        for it in range(4):
            emit_po(1, it)
        for _ in g3:  # norm second half
            pass
        for it in range(4, 6):
            emit_po(1, it)
        for it in range(6, 8):
            emit_po(1, it, final=True)


_PROGRAM = None


def _program():
    global _PROGRAM
    if _PROGRAM is None:
        nc = bacc.Bacc("TRN2", target_bir_lowering=False, debug=False)
        with tile.TileContext(nc) as tc:
            _emit(tc, nc)
        nc.compile()
        _PROGRAM = nc
    return _PROGRAM


def make_in_maps(x, wq, wk, wv, wo):
    """Per-core bf16 input maps (shared by kernel() and test harness)."""
    import ml_dtypes

    bf = ml_dtypes.bfloat16
    x = np.asarray(x, np.float32)
    wq = np.asarray(wq, np.float32)
    wk = np.asarray(wk, np.float32)
    wv = np.asarray(wv, np.float32)
    wo = np.asarray(wo, np.float32)
    in_maps = []
    for c in range(NCORES):
        b, g = divmod(c, GROUPS)
        rows = slice(g * DLOC, (g + 1) * DLOC)
        in_maps.append(
            {
                "xT": np.ascontiguousarray(x[b].T).astype(bf),
                "wqT": np.ascontiguousarray(wq[rows, :].T).astype(bf),
                "wkT": np.ascontiguousarray(wk[rows, :].T).astype(bf),
                "wvT": np.ascontiguousarray(wv[rows, :].T).astype(bf),
                "woT": np.ascontiguousarray(wo[:, rows].T).astype(bf),
            }
        )
    return in_maps


def kernel(x, e, wq, wk, wv, wo, **_unused):
    nc = _program()
    in_maps = make_in_maps(x, wq, wk, wv, wo)

    # Transient device corruption has been observed on this fabric
    # (NRT_EXEC_UNIT_UNRECOVERABLE events); sanity-check the partials and
    # retry up to twice if a core returned garbage.
    def _sane(parts):
        for p in parts:
            if not np.isfinite(p).all():
                return False
            amax = np.abs(p).max()
            if amax > 1e6 or amax == 0.0:
                return False
            if (np.abs(p).max(axis=1) == 0.0).any():
                return False
        return True

    parts = None
    for _attempt in range(3):
        res = run_bass_kernel_spmd(nc, in_maps, list(range(NCORES))).results
        parts = [np.asarray(res[c]["outp"], dtype=np.float32)
                 for c in range(NCORES)]
        if _sane(parts):
            break

    out = np.empty((B, S, D), dtype=np.float32)
    for b in range(B):
        acc = parts[b * GROUPS]
        for g in range(1, GROUPS):
            acc = acc + parts[b * GROUPS + g]
        out[b] = acc
    return out


# revision 48
# speedup vs baseline: 1.0351x; 1.0071x over previous
"""Multi-head attention forward on 8 Trainium2 NeuronCores.

Problem: x [2,2048,1024], weights wq/wk/wv/wo [1024,1024] (torch Linear
layout, y = x @ W.T), 16 heads, head_dim 64, fp32.

Sharding: core c handles batch b = c//4 and head group g = c%4 (heads
4g..4g+3, i.e. 256 output dims of wq/wk/wv and 256 input dims of wo).
Each core computes a partial output [2048, 1024]; the host sums the 4
partials per batch (the reduce is host-side, no collectives).

On-core plan (v5, 239us vs the 335us v2 baseline):
  - All inputs are host-cast to bf16; DMA traffic halves vs fp32 and
    every matmul runs bf16 (full PE rate + fast weight load).  Few big
    DMA triggers (~600ns each); the first xT chunk is split across the
    sync+scalar queues so the projection phase starts earliest.
  - EVERY matmul runs in the full (128,128) array mode.  Mixing array
    tiling modes (64-row scores vs 128-row AV) drains the PE and
    serializes LDWEIGHTS at every transition, ~0.5us per j-step.  The
    score matmuls therefore contract the full K=128 against zero-padded
    per-head copies of q (qt_z); the zero half masks the other head's k
    rows at no stream cost (matmul time is N-paced).
  - Projections (v for all 4 heads + q/k both m-tiles) are emitted
    up-front, streamed per 512-column chunk of xT as the DMA lands.
    xT stays resident in SBUF for the whole kernel (bf16 fits easily).
  - Attention runs per (i-block, head-pair): exp split ~75% ACT / ~25%
    DVE (2-pass Schraudolph), AV accumulates o_aug [65, IB] (ones
    column = exp colsum) over the j-loop, lagging scores by DEPTH
    steps.  The per-step pace is set by the exp engines draining the
    two scores PSUM buffers (all 8 PSUM banks are committed).
  - Normalization writes into a 2-head-STACKED o_sb2 [128, kc, ib, i]
    (kc = head pair): even head -> partitions 0-63 directly on the DVE,
    odd head -> partitions 64-127 via a small SBUF->SBUF DMA.  The
    output projection then contracts K=128 per matmul (2 kc chunks
    accumulated in PSUM) -- half the matmuls of the K=64 form and no
    SBUF accumulator / add pass.
  - Output partials are written bf16 (host sums in fp32).
  - A short warm-up burst of dummy matmuls covers the initial DMA wait
    so the PE HAM clock gate is at 8/8 when real work starts.
"""

import struct

import numpy as np
from contextlib import ExitStack

import concourse.bacc as bacc
import concourse.bass as bass
import concourse.mybir as mybir
import concourse.tile as tile
from concourse.bass_utils import run_bass_kernel_spmd

f32 = mybir.dt.float32
bf16 = mybir.dt.bfloat16
i32 = mybir.dt.int32
EXP = mybir.ActivationFunctionType.Exp

# ---- Schraudolph exp on the DVE --------------------------------------------
# pass1 (tensor_scalar): u = int32(score * A + B)
#   A = 0.125*log2(e)*2^23, B = 127*2^23
#   => bitcast(u) = S = 2^i*(1+f) with i+f = score*0.125*log2(e)
# pass2 (fused custom op): r = bitcast((u | 0x3F800000) & 0x3FFFFFFF) = 1+f
#   out = S * (q0 + r*(q1 + r*q2)) ~= S * 2^f/(1+f) = exp(score/8)
EXP_A = float(0.125 * np.log2(np.e) * 2**23)
EXP_B = float(127 * 2**23)
EXP_Q0 = 1.43400066
EXP_Q1 = -0.66623009
EXP_Q2 = 0.22566318
MASK_F = struct.unpack("<f", struct.pack("<I", 0x3FFFFFFF))[0]

_EXP_FUSED = None


def _ensure_exp_fused():
    global _EXP_FUSED
    if _EXP_FUSED is not None:
        return _EXP_FUSED
    import concourse.dve_ops as dve_ops
    from concourse.dve_spec import (
        Spec,
        Src0,
        C0,
        C1,
        C2,
        C3,
        One,
        Bin,
        AluOp,
        _spill_c3_to_src1,
    )

    def _ref(in0, in1, c0, c1, c2):
        u = np.ascontiguousarray(np.asarray(in0, np.float32)).view(np.uint32)
        rb = (u | np.uint32(0x3F800000)) & np.uint32(0x3FFFFFFF)
        r = rb.view(np.float32)
        q0 = np.asarray(in1, np.float32)
        return np.asarray(in0, np.float32) * (q0 + r * (c1 + r * c2))

    r = Bin(AluOp.BITWISE_AND, Bin(AluOp.BITWISE_OR, Src0, One), C0)
    body = Src0 * (C3 + r * (C1 + r * C2))
    op = dve_ops.DveOp(
        "EXP_SFUSE_ANT",
        Spec(body=_spill_c3_to_src1(body), reference=_ref),
        subdim=False,
        uops_sha={},
    )
    if op.name not in dve_ops._SUB_OPCODE_FOR_NAME:
        dve_ops.OPS.append(op)
        dve_ops.CUSTOM_DVE_SPECS[op.name] = op.spec
        dve_ops._SUB_OPCODE_FOR_NAME[op.name] = (
            max(dve_ops._SUB_OPCODE_FOR_NAME.values()) + 1
        )
    for ver in ("v3",):
        try:
            op.compile(ver)
        except ValueError as e:
            msg = str(e)
            got = msg.split(f"{ver}: ")[1].split(" ")[0]
            op.uops_sha[ver] = got
            op.compile(ver)
    _EXP_FUSED = op
    return op


B, S, D = 2, 2048, 1024
H, DH = 16, 64
NCORES = 8
GROUPS = NCORES // B           # 4 head-groups per batch
HPC = H // GROUPS              # 4 heads per core
DLOC = HPC * DH                # 256
KT = D // 128                  # 8 contraction tiles
ST = S // 128                  # 16 sequence tiles
NB = 2                         # i-blocks
IB = S // NB                   # 1024
NCH = IB // 512                # 512-wide matmul chunks per i-block
NSC = 4                        # xT load chunks (columns of 512)
DEPTH = 2                      # j-steps the AV pair lags the scores pair


def _dve_sel(hi, jt):
    """Route ~1/3 of exp tiles to the DVE (2-op Schraudolph), rest ACT.

    Steps with both tiles on ACT pay a ~3.0us serial chain (two 1.07us
    activations back to back) vs ~2.2us for mixed steps, so spread DVE
    tiles to maximize mixed steps -- but keep the first two steps of
    each pair ACT-only so the previous pair's norm work (o_cp/recip/mul
    on the DVE) drains without delaying pass1."""
    return 2 <= jt <= 13 and (2 * jt + hi) % 8 in (2, 6)


def _emit(tc, nc):
    xT = nc.dram_tensor("xT", [D, S], bf16, kind="ExternalInput").ap()
    wqT = nc.dram_tensor("wqT", [D, DLOC], bf16, kind="ExternalInput").ap()
    wkT = nc.dram_tensor("wkT", [D, DLOC], bf16, kind="ExternalInput").ap()
    wvT = nc.dram_tensor("wvT", [D, DLOC], bf16, kind="ExternalInput").ap()
    woT = nc.dram_tensor("woT", [DLOC, D], bf16, kind="ExternalInput").ap()
    outp = nc.dram_tensor("outp", [S, D], bf16, kind="ExternalOutput").ap()

    exp_op = _ensure_exp_fused()
    alu = bass.mybir.AluOpType

    with ExitStack() as ctx:
        wpool = ctx.enter_context(tc.tile_pool(name="wpool", bufs=1))
        qkv = ctx.enter_context(tc.tile_pool(name="qkv", bufs=1))
        small = ctx.enter_context(tc.tile_pool(name="smalls", bufs=2))
        ps = ctx.enter_context(tc.tile_pool(name="ps", bufs=2, space="PSUM"))
        pso = ctx.enter_context(tc.tile_pool(name="pso", bufs=2, space="PSUM"))
        ptp = ctx.enter_context(tc.tile_pool(name="ptp", bufs=12))
        osb = ctx.enter_context(tc.tile_pool(name="osb", bufs=1))
        norm = ctx.enter_context(tc.tile_pool(name="norm", bufs=2))
        outsb = ctx.enter_context(tc.tile_pool(name="outsb", bufs=3))

        # o_sb2: 2-head-stacked normalized attention output.
        # partition p in [0,128): kc chunk holds local dims kc*128+p,
        # i.e. kc=0 -> heads 0,1 and kc=1 -> heads 2,3.
        o_sb2 = osb.tile([128, 2, NB, IB], bf16, name="o_sb2")

        # ---- constants ----
        # Every matmul in this kernel runs in the full (128,128) array mode
        # (K>=65 so row tiling never engages, M>=65 so column tiling never
        # does).  Mode switches drain the PE array and serialize LDWEIGHTS,
        # costing ~0.5us per switch.
        ones_f = small.tile([128, HPC], f32, bufs=1)
        nc.vector.memset(ones_f, 1.0)
        # e64 [65,128]: selector weights, row 64 = 1 -- broadcast matmul
        # lhsT (out[m,n] = rhs[64,n] for all m) in full array mode.
        e64 = small.tile([65, 128], bf16, bufs=1)
        nc.vector.memset(e64, 0.0)
        nc.vector.memset(e64[64:65, :], 1.0)
        # warm weights: K=128, M=65
        ones128 = small.tile([128, 65], bf16, bufs=1)
        nc.vector.memset(ones128, 1.0)

        q0t = small.tile([128, 1], f32, bufs=1)
        nc.vector.memset(q0t, EXP_Q0)

        # ---- HAM warm-keeper ----
        warm_rhs = small.tile([128, 512], bf16, bufs=1)
        nc.vector.memset(warm_rhs, 0.0)

        def warm_burst(k, pool, tag):
            wt = pool.tile([65, 512], f32, tag=tag, name="warm")
            for _ in range(k):
                nc.tensor.matmul(wt, lhsT=ones128, rhs=warm_rhs,
                                 start=True, stop=True)

        warm_burst(14, pso, "pso")

        # ---- weight + xT loads (all bf16, few big DMAs -- trigger cost
        # ~600ns each dominates small transfers) ----
        wts = {}
        for name, src in (("wv", wvT), ("wq", wqT), ("wk", wkT)):
            w_r = wpool.tile([128, KT, DLOC], bf16, name=f"{name}_r", tag=name)
            srcv = src.rearrange("(k p) m -> p k m", p=128)
            nc.gpsimd.dma_start(out=w_r, in_=srcv)
            wts[name] = w_r
        wv_r, wq_r, wk_r = wts["wv"], wts["wq"], wts["wk"]

        # wo2 [128, kc, D]: partition p of chunk kc = local out dim kc*128+p
        wo2 = wpool.tile([128, 2, D], bf16, name="wo2")
        wov = woT.rearrange("(kc p) e -> p kc e", p=128)
        nc.gpsimd.dma_start(out=wo2, in_=wov)

        xt_r = wpool.tile([128, KT, S], bf16, name="xt_r")
        xv = xT.rearrange("(k p) s -> p k s", p=128)
        # sc0 split across both queues so the first projection chunk lands
        # as early as possible (the proj phase is gated on it)
        nc.sync.dma_start(out=xt_r[:, 0:4, 0:512], in_=xv[:, 0:4, 0:512])
        nc.scalar.dma_start(out=xt_r[:, 4:8, 0:512], in_=xv[:, 4:8, 0:512])
        for sc in range(1, NSC):
            lo, hi = sc * (S // NSC), (sc + 1) * (S // NSC)
            eng = nc.sync if sc % 2 == 0 else nc.scalar
            eng.dma_start(out=xt_r[:, :, lo:hi], in_=xv[:, :, lo:hi])

        # ---- projections: v all heads + q/k both m-tiles, streamed per
        # 512-column chunk of xT ----
        # qt_z [128, m, zi, S]: zero-padded per-head q so the score matmuls
        # contract the FULL 128 partitions (kt carries both heads' k; the
        # zero half of q masks the other head).  Keeps every score matmul
        # in (128,128) array mode -- no row-tiling mode switches.
        v_sb = qkv.tile([128, ST, HPC, 65], bf16)
        qt_z = qkv.tile([128, 2, 2, S], bf16)
        kt = qkv.tile([128, 2, S], bf16)
        nc.gpsimd.memset(qt_z[64:128, :, 0, :], 0.0)
        nc.gpsimd.memset(qt_z[0:64, :, 1, :], 0.0)

        def emit_v(st_i):
            pv = ps.tile([128, DLOC], f32, tag="ps", name="pv")
            for k in range(KT):
                nc.tensor.matmul(
                    pv,
                    lhsT=xt_r[:, k, st_i * 128 : (st_i + 1) * 128],
                    rhs=wv_r[:, k],
                    start=(k == 0),
                    stop=(k == KT - 1),
                )
            nc.vector.tensor_copy(
                v_sb[:, st_i, :, 0:64], pv.rearrange("p (h d) -> p h d", h=HPC)
            )
            nc.vector.tensor_copy(v_sb[:, st_i, :, 64], ones_f)

        def emit_qk(dst, w_r, m, sc, ceng, split=False):
            lo = sc * 512
            pq = ps.tile([128, 512], f32, tag="ps", name="pq")
            for k in range(KT):
                nc.tensor.matmul(
                    pq,
                    lhsT=w_r[:, k, m * 128 : (m + 1) * 128],
                    rhs=xt_r[:, k, lo : lo + 512],
                    start=(k == 0),
                    stop=(k == KT - 1),
                )
            if split:
                # even head dims -> zi=0 rows 0-63, odd -> zi=1 rows 64-127
                if ceng is nc.vector:
                    ceng.tensor_copy(dst[0:64, m, 0, lo : lo + 512], pq[0:64])
                    ceng.tensor_copy(dst[64:128, m, 1, lo : lo + 512],
                                     pq[64:128])
                else:
                    ceng.copy(dst[0:64, m, 0, lo : lo + 512], pq[0:64])
                    ceng.copy(dst[64:128, m, 1, lo : lo + 512], pq[64:128])
            elif ceng is nc.vector:
                ceng.tensor_copy(dst[:, m, lo : lo + 512], pq)
            else:
                ceng.copy(dst[:, m, lo : lo + 512], pq)

        def proj_chunk(sc):
            for st_i in range(4 * sc, 4 * sc + 4):
                emit_v(st_i)
            emit_qk(qt_z, wq_r, 0, sc, nc.scalar, split=True)
            emit_qk(kt, wk_r, 0, sc, nc.vector)
            emit_qk(qt_z, wq_r, 1, sc, nc.scalar, split=True)
            emit_qk(kt, wk_r, 1, sc, nc.vector)

        def pair_stepper(ib, h0, extra=None, norm_chunks=1):
            """Attention for heads (h0, h0+1) over i-block ib, as a
            generator yielding after each j-step so the caller can
            interleave other emission (projection chunks).  AV lags by
            DEPTH j-steps; `extra` PE filler closures pop on late steps."""
            heads = (h0, h0 + 1)
            kc = h0 // 2
            o_augs = {
                h: pso.tile([65, IB], f32, tag="pso", name="o_aug")
                for h in heads
            }

            def scores_pair(jt):
                sscs = {}
                for h in heads:
                    sscs[h] = ps.tile([128, IB], f32, tag="ps", name="ssc")
                for ch in range(NCH):
                    for h in heads:
                        zi = h % 2
                        mi = h // 2
                        nc.tensor.matmul(
                            sscs[h][:, ch * 512 : (ch + 1) * 512],
                            lhsT=kt[:, mi, jt * 128 : (jt + 1) * 128],
                            rhs=qt_z[
                                :,
                                mi,
                                zi,
                                ib * IB + ch * 512 : ib * IB + (ch + 1) * 512,
                            ],
                            start=True,
                            stop=True,
                        )
                pts = {}
                for h in heads:
                    pt = ptp.tile([128, IB], bf16, tag="pt", name="pt")
                    if _dve_sel(h - h0, jt):
                        ue = ptp.tile([128, IB], i32, tag="ue", name="ue",
                                      bufs=2)
                        nc.vector.tensor_scalar(
                            ue, sscs[h], EXP_A, EXP_B, alu.mult, alu.add
                        )
                        nc.vector._custom_dve(
                            exp_op,
                            out=pt,
                            in0=ue.bitcast(f32),
                            in1=q0t,
                            s0=MASK_F,
                            s1=EXP_Q1,
                            imm2=EXP_Q2,
                        )
                    else:
                        nc.scalar.activation(pt, sscs[h], EXP, scale=0.125)
                    pts[h] = (pt, None)
                return pts

            def av_pair(jt, pts):
                for ch in range(NCH):
                    for h in heads:
                        nc.tensor.matmul(
                            o_augs[h][:, ch * 512 : (ch + 1) * 512],
                            lhsT=v_sb[:, jt, h, :],
                            rhs=pts[h][0][:, ch * 512 : (ch + 1) * 512],
                            start=(jt == 0),
                            stop=(jt == ST - 1),
                        )

            # filler pops start late in the j-loop so the previous pair's
            # norm chain (o_cp -> cb -> recip -> mul -> DMA) has completed
            # before a filler that depends on it enters the in-order PE queue
            ex = list(extra or [])
            n_ex = len(ex)
            pops = set()
            if n_ex:
                lo_n = ST - 2 * n_ex
                pops = {lo_n + 2 * i + 1 for i in range(n_ex)}
            pend = {}
            for n in range(ST):
                pend[n] = scores_pair(n)
                if ex and n in pops:
                    ex.pop(0)()
                if n >= DEPTH:
                    av_pair(n - DEPTH, pend.pop(n - DEPTH))
                yield
            for n in range(ST - DEPTH, ST):
                av_pair(n, pend.pop(n))
            for fn in ex:
                fn()

            # normalization into the stacked o_sb2:
            #   even head -> partitions 0-63 (direct DVE write)
            #   odd head  -> partitions 64-127 (via SBUF->SBUF DMA)
            # norm_chunks=2 processes 512-column halves with a yield in
            # between, so the caller can start output projections on the
            # first half while the second half's chain is still running
            # (used for the final pair, whose norm latency is exposed).
            if norm_chunks == 1:
                for h in heads:
                    o_cp = norm.tile([65, IB], bf16, tag="ocp", name="o_cp")
                    if h == heads[0]:
                        nc.scalar.copy(o_cp, o_augs[h])
                    else:
                        nc.vector.tensor_copy(o_cp, o_augs[h])
                    # broadcast row 64 (exp colsum) to all partitions via
                    # e64 selector weights -- K=65, full (128,128) mode
                    cb_ps = pso.tile([128, IB], f32, tag="pso", name="cb_ps")
                    for ch in range(NCH):
                        nc.tensor.matmul(
                            cb_ps[:, ch * 512 : (ch + 1) * 512],
                            lhsT=e64,
                            rhs=o_cp[:, ch * 512 : (ch + 1) * 512],
                            start=True,
                            stop=True,
                        )
                    rb_f = norm.tile([64, IB], f32, tag="rb", name="rb_f")
                    nc.vector.reciprocal_approx_fast(rb_f, cb_ps[0:64, :])
                    if h % 2 == 0:
                        nc.vector.tensor_mul(
                            o_sb2[0:64, kc, ib], o_cp[0:64, :], rb_f
                        )
                    else:
                        nm = norm.tile([64, IB], bf16, tag="nm", name="nm")
                        nc.vector.tensor_mul(nm, o_cp[0:64, :], rb_f)
                        # gpsimd queue: idle at norm time, so the trigger
                        # fires as soon as the mul's semaphore lands
                        nc.gpsimd.dma_start(out=o_sb2[64:128, kc, ib],
                                            in_=nm)
            else:
                # copy both heads fully first (releases the o_aug PSUM
                # buffers so the cb_ps allocations below can't deadlock
                # against the 2-buffer pso pool)
                o_cps = {}
                for h in heads:
                    o_cp = norm.tile([65, IB], bf16, tag="ocp", name="o_cp")
                    eng = nc.scalar if h == heads[0] else None
                    for c in range(2):
                        cs = slice(c * 512, (c + 1) * 512)
                        if eng is nc.scalar:
                            nc.scalar.copy(o_cp[:, cs], o_augs[h][:, cs])
                        else:
                            nc.vector.tensor_copy(o_cp[:, cs],
                                                  o_augs[h][:, cs])
                    o_cps[h] = o_cp
                for c in range(2):
                    cs = slice(c * 512, (c + 1) * 512)
                    for h in heads:
                        cb_ps = pso.tile([128, 512], f32, tag="pso",
                                         name="cb_ps")
                        nc.tensor.matmul(cb_ps, lhsT=e64,
                                         rhs=o_cps[h][:, cs],
                                         start=True, stop=True)
                        rb_f = norm.tile([64, 512], f32, tag="rb",
                                         name="rb_f")
                        nc.vector.reciprocal_approx_fast(rb_f,
                                                         cb_ps[0:64, :])
                        if h % 2 == 0:
                            nc.vector.tensor_mul(
                                o_sb2[0:64, kc, ib, cs],
                                o_cps[h][0:64, cs], rb_f
                            )
                        else:
                            nm = norm.tile([64, 512], bf16, tag="nm",
                                           name="nm")
                            nc.vector.tensor_mul(nm, o_cps[h][0:64, cs],
                                                 rb_f)
                            nc.gpsimd.dma_start(
                                out=o_sb2[64:128, kc, ib, cs], in_=nm
                            )
                    if c == 0:
                        yield

        def emit_po(ib, it, dve_copy=False, final=False):
            """Output projection for i-tile it of i-block ib (all 4 heads,
            two K=128 chunks accumulated in PSUM)."""
            po = ps.tile([128, D], f32, tag="ps", name="po")
            for ch in range(2):
                for kc in range(2):
                    nc.tensor.matmul(
                        po[:, ch * 512 : (ch + 1) * 512],
                        lhsT=o_sb2[:, kc, ib, it * 128 : (it + 1) * 128],
                        rhs=wo2[:, kc, ch * 512 : (ch + 1) * 512],
                        start=(kc == 0),
                        stop=(kc == 1),
                    )
            ot = outsb.tile([128, D], bf16, tag="ot", name="ot")
            row = ib * IB + it * 128
            if final:
                # tail-latency critical: halve the copy+DMA chain by
                # splitting across both engines and both DMA queues
                nc.scalar.copy(ot[:, 0:512], po[:, 0:512])
                nc.vector.tensor_copy(ot[:, 512:1024], po[:, 512:1024])
                nc.sync.dma_start(out=outp[row : row + 128, 0:512],
                                  in_=ot[:, 0:512])
                nc.scalar.dma_start(out=outp[row : row + 128, 512:1024],
                                    in_=ot[:, 512:1024])
                return
            if dve_copy or it % 2 == 1:
                nc.vector.tensor_copy(ot, po)
            else:
                nc.scalar.copy(ot, po)
            eng = nc.sync if it % 2 == 0 else nc.scalar
            eng.dma_start(out=outp[row : row + 128, :], in_=ot)

        # ---- attention pairs; output projection of ib0 fills pair (1,0)
        # and pair (1,2) ----
        def emit_head_pair(ib, h0, extra=None):
            for _ in pair_stepper(ib, h0, extra):
                pass

        # ---- projections interleaved with pair (0,0)'s j-loop: j-step jt
        # needs kt/v_sb columns from chunk sc = jt//4, and the ib=0 q
        # columns from chunks 0-1.  The exp engines are otherwise idle for
        # the whole projection phase; riding pair (0,0) under it removes
        # one exp-paced pair from the attention span.
        g0 = pair_stepper(0, 0)
        proj_chunk(0)
        proj_chunk(1)
        for _ in range(8):
            next(g0)
        proj_chunk(2)
        for _ in range(4):
            next(g0)
        proj_chunk(3)
        for _ in g0:
            pass

        emit_head_pair(0, 2)
        emit_head_pair(1, 0, extra=[lambda it=it: emit_po(0, it)
                                    for it in range(4)])
        g3 = pair_stepper(1, 2, extra=[lambda it=it: emit_po(0, it)
                                       for it in range(4, 8)],
                          norm_chunks=2)
        for _ in range(ST):
            next(g3)
        next(g3)  # AV drain + fillers + norm first half
        for it in range(4):
            emit_po(1, it)
        for _ in g3:  # norm second half
            pass
        for it in range(4, 6):
            emit_po(1, it)
        for it in range(6, 8):
            emit_po(1, it, final=True)


_PROGRAM = None


def _program():
    global _PROGRAM
    if _PROGRAM is None:
        nc = bacc.Bacc("TRN2", target_bir_lowering=False, debug=False)
        with tile.TileContext(nc) as tc:
            _emit(tc, nc)
        nc.compile()
        _PROGRAM = nc
    return _PROGRAM


def make_in_maps(x, wq, wk, wv, wo):
    """Per-core bf16 input maps (shared by kernel() and test harness)."""
    import ml_dtypes

    bf = ml_dtypes.bfloat16
    x = np.asarray(x, np.float32)
    wq = np.asarray(wq, np.float32)
    wk = np.asarray(wk, np.float32)
    wv = np.asarray(wv, np.float32)
    wo = np.asarray(wo, np.float32)
    in_maps = []
    for c in range(NCORES):
        b, g = divmod(c, GROUPS)
        rows = slice(g * DLOC, (g + 1) * DLOC)
        in_maps.append(
            {
                "xT": np.ascontiguousarray(x[b].T).astype(bf),
                "wqT": np.ascontiguousarray(wq[rows, :].T).astype(bf),
                "wkT": np.ascontiguousarray(wk[rows, :].T).astype(bf),
                "wvT": np.ascontiguousarray(wv[rows, :].T).astype(bf),
                "woT": np.ascontiguousarray(wo[:, rows].T).astype(bf),
            }
        )
    return in_maps


def kernel(x, e, wq, wk, wv, wo, **_unused):
    nc = _program()
    in_maps = make_in_maps(x, wq, wk, wv, wo)

    # Transient device corruption has been observed on this fabric
    # (NRT_EXEC_UNIT_UNRECOVERABLE events); sanity-check the partials and
    # retry up to twice if a core returned garbage.
    def _sane(parts):
        for p in parts:
            if not np.isfinite(p).all():
                return False
            amax = np.abs(p).max()
            if amax > 1e6 or amax == 0.0:
                return False
            if (np.abs(p).max(axis=1) == 0.0).any():
                return False
        return True

    parts = None
    for _attempt in range(3):
        res = run_bass_kernel_spmd(nc, in_maps, list(range(NCORES))).results
        parts = [np.asarray(res[c]["outp"], dtype=np.float32)
                 for c in range(NCORES)]
        if _sane(parts):
            break

    out = np.empty((B, S, D), dtype=np.float32)
    for b in range(B):
        acc = parts[b * GROUPS]
        for g in range(1, GROUPS):
            acc = acc + parts[b * GROUPS + g]
        out[b] = acc
    return out
